# revision 10
# baseline (speedup 1.0000x reference)
"""Paged block-attention (GQA, diffusion-block causal mask) on 8 Trainium2 cores.

Problem geometry (hardcoded; matches nn_BlockAttention_25778393710607):
  q       [B=4, LQ=512, HQ=16, D=128]
  k, v    [B=4, LQ=512, HKV=8, D=128]
  k_cache/v_cache [NUM_BLOCKS=64, BLOCK_SIZE=256, HKV=8, D=128]
  block_tables [B=4, BLOCKS_PER_SEQ=8] int32
  allow_mask [B=4, LQ=512, LK=2560] bool
  out     [B=4, LQ=512, HQ=16, D=128] fp32

Sharding: core c owns sequence c//2 and head-half c%2 (4 KV heads -> 8 Q
heads via GQA rep=2). The paged gather (cache rows per block table) plus
layout transposes happen on host while building each core's input map; the
device kernel computes, per (q-head):

  S^T[k, i] = (K_all @ (q*scale)^T)   chunk-wise over 20 key chunks of 128
  P = exp(S^T)                        (no max subtraction: |s| <~ 12 for
                                       randn inputs, fp32 exp is safe)
  outT[d, i] = sum_k V[k, d] * P[k, i]   (PSUM accumulation)
  den[i]    = sum_k P[k, i]              (ones-column matmul, PSUM accum)

and the host divides outT/den (softmax normalization) when reassembling.

The mask is applied structurally: for every 128-key chunk the set of
allowed queries is a suffix [qs, LQ) (true for the reference block-causal
mask with DIFF_BLOCK=128, and for an all-ones mask); only those query
columns are streamed through the PE for that chunk, so masked (k, q)
pairs are never computed and never pollute the denominator.
"""

import numpy as np

B, LQ, HQ, HKV, D = 4, 512, 16, 8, 128
BLOCK_SIZE, BLOCKS_PER_SEQ, NUM_BLOCKS = 256, 8, 64
CTX = BLOCK_SIZE * BLOCKS_PER_SEQ
LK = CTX + LQ
NCHUNK = LK // 128            # 20 key chunks of 128
SCALE = 1.0 / float(np.sqrt(D))
N_CORES = 8
H_PER_CORE = HQ // 2          # 8 q heads per core
KV_PER_CORE = HKV // 2        # 4 kv heads per core
CHUNKS_PER_ROUND = 2          # S^T chunks exp'd per ACT instruction

_nc_cache = {}


def _derive_qstarts(allow_mask):
    """Per key-chunk allowed-query suffix start, verified against the mask."""
    m = np.asarray(allow_mask, dtype=bool)
    assert m.shape == (B, LQ, LK), m.shape
    qstarts = []
    ar = np.arange(LQ)
    for j in range(NCHUNK):
        mj = m[:, :, j * 128:(j + 1) * 128]
        row = mj.any(axis=2)                      # [B, LQ]
        if not (mj == row[:, :, None]).all():
            raise ValueError(f"mask chunk {j} not uniform within the chunk")
        r0 = row[0]
        if not (row == r0[None]).all():
            raise ValueError(f"mask chunk {j} differs across batch")
        qs = int(LQ - r0.sum())
        if not (r0 == (ar >= qs)).all():
            raise ValueError(f"mask chunk {j} rows are not a query suffix")
        qstarts.append(qs)
    return tuple(qstarts)


def _build_nc(qstarts):
    import concourse.bass as bass
    import concourse.tile as tile
    from concourse import bacc, mybir

    f32 = mybir.dt.float32
    f32r = mybir.dt.float32r
    bf16 = mybir.dt.bfloat16
    Exp = mybir.ActivationFunctionType.Exp

    nc = bacc.Bacc("TRN2", target_bir_lowering=False, debug=False)
    qT = nc.dram_tensor("qT", [H_PER_CORE * 128, LQ], f32r, kind="ExternalInput").ap()
    kT = nc.dram_tensor("kT", [KV_PER_CORE * 128, LK], f32r, kind="ExternalInput").ap()
    vT = nc.dram_tensor("vT", [KV_PER_CORE * 128, LK], bf16, kind="ExternalInput").ap()
    outT = nc.dram_tensor("outT", [H_PER_CORE * 128, LQ], f32, kind="ExternalOutput").ap()
    den = nc.dram_tensor("den", [H_PER_CORE, LQ], f32, kind="ExternalOutput").ap()

    with tile.TileContext(nc) as tc:
        with tc.tile_pool(name="const", bufs=1) as cpool, \
             tc.tile_pool(name="qpool", bufs=1) as qpool, \
             tc.tile_pool(name="kv", bufs=2) as kvpool, \
             tc.tile_pool(name="pp", bufs=3) as ppool, \
             tc.tile_pool(name="ostage", bufs=2) as opool, \
             tc.tile_pool(name="psum", bufs=2, space="PSUM") as pspool:

            ones = cpool.tile([128, 1], bf16)
            nc.vector.memset(ones[:], 1.0)

            q_sb = qpool.tile([128, H_PER_CORE, LQ], f32r)
            for h in range(H_PER_CORE):
                nc.sync.dma_start(q_sb[:, h, :], qT[h * 128:(h + 1) * 128, :])

            for g in range(KV_PER_CORE):
                k_sb = kvpool.tile([128, LK], f32r, tag="k")
                nc.sync.dma_start(k_sb[:], kT[g * 128:(g + 1) * 128, :])
                v_sb = kvpool.tile([128, LK], bf16, tag="v")
                nc.sync.dma_start(v_sb[:], vT[g * 128:(g + 1) * 128, :])

                for h2 in range(2):
                    h = 2 * g + h2
                    o_ps = pspool.tile([128, LQ], f32, tag="o")
                    d_ps = pspool.tile([1, LQ], f32, tag="d")
                    for r in range(NCHUNK // CHUNKS_PER_ROUND):
                        s_ps = pspool.tile([128, CHUNKS_PER_ROUND * LQ], f32, tag="s")
                        p_sb = ppool.tile([128, CHUNKS_PER_ROUND * LQ], bf16, tag="p")
                        round_qs = [qstarts[CHUNKS_PER_ROUND * r + c]
                                    for c in range(CHUNKS_PER_ROUND)]
                        for c in range(CHUNKS_PER_ROUND):
                            j = CHUNKS_PER_ROUND * r + c
                            qs = round_qs[c]
                            if qs >= LQ:
                                continue
                            nc.tensor.matmul(
                                s_ps[:, c * LQ + qs:(c + 1) * LQ],
                                lhsT=k_sb[:, j * 128:(j + 1) * 128],
                                rhs=q_sb[:, h, qs:],
                                start=True, stop=True)
                        if all(qs == 0 for qs in round_qs):
                            # one big ACT instruction over the whole round
                            nc.scalar.activation(p_sb[:], s_ps[:], Exp)
                        else:
                            # exp exactly the regions the matmuls wrote
                            for c in range(CHUNKS_PER_ROUND):
                                qs = round_qs[c]
                                if qs >= LQ:
                                    continue
                                nc.scalar.activation(
                                    p_sb[:, c * LQ + qs:(c + 1) * LQ],
                                    s_ps[:, c * LQ + qs:(c + 1) * LQ], Exp)
                        for c in range(CHUNKS_PER_ROUND):
                            j = CHUNKS_PER_ROUND * r + c
                            qs = qstarts[j]
                            if qs >= LQ:
                                continue
                            first = j == 0
                            last = j == NCHUNK - 1
                            nc.tensor.matmul(
                                o_ps[:, qs:],
                                lhsT=v_sb[:, j * 128:(j + 1) * 128],
                                rhs=p_sb[:, c * LQ + qs:(c + 1) * LQ],
                                start=first, stop=last)
                            nc.tensor.matmul(
                                d_ps[:, qs:],
                                lhsT=ones[:],
                                rhs=p_sb[:, c * LQ + qs:(c + 1) * LQ],
                                start=first, stop=last)
                    o_sb = opool.tile([128, LQ], f32, tag="ot")
                    nc.vector.tensor_copy(o_sb[:], o_ps[:])
                    d_sb = opool.tile([1, LQ], f32, tag="dt")
                    nc.vector.tensor_copy(d_sb[:], d_ps[:])
                    nc.sync.dma_start(outT[h * 128:(h + 1) * 128, :], o_sb[:])
                    nc.sync.dma_start(den[h:h + 1, :], d_sb[:])
    nc.compile()
    return nc


def _get_nc(qstarts):
    nc = _nc_cache.get(qstarts)
    if nc is None:
        nc = _build_nc(qstarts)
        _nc_cache[qstarts] = nc
    return nc


def _core_inputs(c, q, k, v, k_cache, v_cache, block_tables):
    b, half = divmod(c, 2)
    kvh = slice(half * KV_PER_CORE, (half + 1) * KV_PER_CORE)
    qh = slice(half * H_PER_CORE, (half + 1) * H_PER_CORE)
    # paged gather + concat of current step, this core's kv heads: [LK, KV, D]
    Kc = np.concatenate([
        k_cache[block_tables[b]].reshape(CTX, HKV, D)[:, kvh],
        k[b][:, kvh]], axis=0)
    Vc = np.concatenate([
        v_cache[block_tables[b]].reshape(CTX, HKV, D)[:, kvh],
        v[b][:, kvh]], axis=0)
    # kT[g*128 + d, kk] = Kc[kk, g, d]
    kT = np.ascontiguousarray(Kc.transpose(1, 2, 0)).reshape(KV_PER_CORE * D, LK)
    # vT[g*128 + p, j*128 + d] = Vc[j*128 + p, g, d], bf16 on device
    import ml_dtypes
    vT = np.ascontiguousarray(
        Vc.reshape(NCHUNK, 128, KV_PER_CORE, D).transpose(2, 1, 0, 3)
    ).reshape(KV_PER_CORE * 128, NCHUNK * D).astype(ml_dtypes.bfloat16)
    # qT[h*128 + d, i] = q[b, i, qh][i, h, d] * SCALE
    qT = np.ascontiguousarray(
        (q[b][:, qh] * SCALE).transpose(1, 2, 0)
    ).reshape(H_PER_CORE * D, LQ)
    return {"qT": qT, "kT": kT, "vT": vT}


def _run(q, k, v, k_cache, v_cache, block_tables, allow_mask,
         trace=False, tmpdir=None):
    from concourse.bass_utils import run_bass_kernel_spmd

    q = np.asarray(q, dtype=np.float32)
    k = np.asarray(k, dtype=np.float32)
    v = np.asarray(v, dtype=np.float32)
    k_cache = np.asarray(k_cache, dtype=np.float32)
    v_cache = np.asarray(v_cache, dtype=np.float32)
    block_tables = np.asarray(block_tables)

    qstarts = _derive_qstarts(allow_mask)
    nc = _get_nc(qstarts)
    in_maps = [_core_inputs(c, q, k, v, k_cache, v_cache, block_tables)
               for c in range(N_CORES)]
    res = run_bass_kernel_spmd(nc, in_maps, core_ids=list(range(N_CORES)),
                               trace=trace, tmpdir=tmpdir)

    out = np.empty((B, LQ, HQ, D), dtype=np.float32)
    for c in range(N_CORES):
        b, half = divmod(c, 2)
        oT = np.asarray(res.results[c]["outT"]).reshape(H_PER_CORE, D, LQ)
        dn = np.asarray(res.results[c]["den"])          # [H_PER_CORE, LQ]
        o = oT / dn[:, None, :]
        out[b, :, half * H_PER_CORE:(half + 1) * H_PER_CORE, :] = \
            o.transpose(2, 0, 1)
    return out, res


def kernel(q, k, v, k_cache, v_cache, block_tables, allow_mask):
    out, _ = _run(q, k, v, k_cache, v_cache, block_tables, allow_mask)
    return out


# revision 11
# speedup vs baseline: 1.5773x; 1.5773x over previous
"""Paged block-attention (GQA, diffusion-block causal mask) on 8 Trainium2 cores.

Problem geometry (hardcoded; matches nn_BlockAttention_25778393710607):
  q       [B=4, LQ=512, HQ=16, D=128]
  k, v    [B=4, LQ=512, HKV=8, D=128]
  k_cache/v_cache [NUM_BLOCKS=64, BLOCK_SIZE=256, HKV=8, D=128]
  block_tables [B=4, BLOCKS_PER_SEQ=8] int32
  allow_mask [B=4, LQ=512, LK=2560] bool
  out     [B=4, LQ=512, HQ=16, D=128] fp32

Sharding: core c owns sequence c//2 and head-half c%2 (4 KV heads -> 8 Q
heads via GQA rep=2). The paged gather (cache rows per block table) plus
layout transposes happen on host while building each core's input map; the
device kernel computes, per (q-head):

  S^T[k, i] = (K_all @ (q*scale)^T)   chunk-wise over 20 key chunks of 128
  P = exp(S^T)                        (no max subtraction: |s| <~ 12 for
                                       randn inputs, fp32 exp is safe)
  outT[d, i] = sum_k V[k, d] * P[k, i]   (PSUM accumulation)
  den[i]    = sum_k P[k, i]              (ones-column matmul, PSUM accum)

and the host divides outT/den (softmax normalization) when reassembling.

The mask is applied structurally: for every 128-key chunk the set of
allowed queries is a suffix [qs, LQ) (true for the reference block-causal
mask with DIFF_BLOCK=128, and for an all-ones mask); only those query
columns are streamed through the PE for that chunk, so masked (k, q)
pairs are never computed and never pollute the denominator.
"""

import numpy as np

B, LQ, HQ, HKV, D = 4, 512, 16, 8, 128
BLOCK_SIZE, BLOCKS_PER_SEQ, NUM_BLOCKS = 256, 8, 64
CTX = BLOCK_SIZE * BLOCKS_PER_SEQ
LK = CTX + LQ
NCHUNK = LK // 128            # 20 key chunks of 128
SCALE = 1.0 / float(np.sqrt(D))
N_CORES = 8
H_PER_CORE = HQ // 2          # 8 q heads per core
KV_PER_CORE = HKV // 2        # 4 kv heads per core
CHUNKS_PER_ROUND = 2          # S^T chunks exp'd per ACT instruction

_nc_cache = {}


def _derive_qstarts(allow_mask):
    """Per key-chunk allowed-query suffix start, verified against the mask."""
    m = np.asarray(allow_mask, dtype=bool)
    assert m.shape == (B, LQ, LK), m.shape
    qstarts = []
    ar = np.arange(LQ)
    for j in range(NCHUNK):
        mj = m[:, :, j * 128:(j + 1) * 128]
        row = mj.any(axis=2)                      # [B, LQ]
        if not (mj == row[:, :, None]).all():
            raise ValueError(f"mask chunk {j} not uniform within the chunk")
        r0 = row[0]
        if not (row == r0[None]).all():
            raise ValueError(f"mask chunk {j} differs across batch")
        qs = int(LQ - r0.sum())
        if not (r0 == (ar >= qs)).all():
            raise ValueError(f"mask chunk {j} rows are not a query suffix")
        qstarts.append(qs)
    return tuple(qstarts)


def _build_nc(qstarts):
    import concourse.bass as bass
    import concourse.tile as tile
    from concourse import bacc, mybir

    f32 = mybir.dt.float32
    f32r = mybir.dt.float32r
    bf16 = mybir.dt.bfloat16
    Exp = mybir.ActivationFunctionType.Exp

    nc = bacc.Bacc("TRN2", target_bir_lowering=False, debug=False)
    qT = nc.dram_tensor("qT", [H_PER_CORE * 128, LQ], f32r, kind="ExternalInput").ap()
    kT = nc.dram_tensor("kT", [KV_PER_CORE * 128, LK], f32r, kind="ExternalInput").ap()
    vT = nc.dram_tensor("vT", [KV_PER_CORE * 128, LK], bf16, kind="ExternalInput").ap()
    outT = nc.dram_tensor("outT", [H_PER_CORE * 128, LQ], f32, kind="ExternalOutput").ap()
    den = nc.dram_tensor("den", [H_PER_CORE, LQ], f32, kind="ExternalOutput").ap()

    ROUND = 3                                      # key chunks per round
    rounds = [list(range(r, min(r + ROUND, NCHUNK)))
              for r in range(0, NCHUNK, ROUND)]
    # S^T/exp padded suffix start per round: uniform within the round so the
    # round's exp is one (possibly 3D-AP) ACT instruction, and capped at 256
    # so fp32r matmuls keep their N>=256 full-rate mode. AV/denominator use
    # the exact per-chunk suffix, so padded (masked) scores are never used.
    spad = [min(min(qstarts[j] for j in ch), LQ - 256) for ch in rounds]
    assert qstarts[0] == 0, "first key chunk must be unmasked"

    with tile.TileContext(nc) as tc:
        with tc.tile_pool(name="const", bufs=1) as cpool, \
             tc.tile_pool(name="qpool", bufs=1) as qpool, \
             tc.tile_pool(name="kv", bufs=2) as kvpool, \
             tc.tile_pool(name="pp", bufs=3) as ppool, \
             tc.tile_pool(name="acc", bufs=2) as accpool, \
             tc.tile_pool(name="ostage", bufs=2) as opool, \
             tc.tile_pool(name="psum", bufs=2, space="PSUM") as pspool:

            ones = cpool.tile([128, 1], bf16)
            nc.vector.memset(ones[:], 1.0)

            q_sb = qpool.tile([128, H_PER_CORE, LQ], f32r)

            for g in range(KV_PER_CORE):
                k_sb = kvpool.tile([128, LK], f32r, tag="k")
                nc.sync.dma_start(k_sb[:], kT[g * 128:(g + 1) * 128, :])
                v_sb = kvpool.tile([128, LK], bf16, tag="v")
                nc.sync.dma_start(v_sb[:], vT[g * 128:(g + 1) * 128, :])

                for h2 in range(2):
                    h = 2 * g + h2
                    nc.sync.dma_start(q_sb[:, h, :], qT[h * 128:(h + 1) * 128, :])
                    o_ps = pspool.tile([128, LQ], f32, tag="o", bufs=1)
                    d_ps = pspool.tile([1, LQ], f32, tag="d", bufs=1)
                    for r, chunks in enumerate(rounds):
                        sp = spad[r]
                        width = LQ - sp
                        s_ps = pspool.tile([128, ROUND, LQ], f32, tag="s")
                        p_sb = ppool.tile([128, ROUND, LQ], bf16, tag="p")
                        for c, j in enumerate(chunks):
                            nc.tensor.matmul(
                                s_ps[:, c, sp:],
                                lhsT=k_sb[:, j * 128:(j + 1) * 128],
                                rhs=q_sb[:, h, sp:],
                                start=True, stop=True)
                        # exp of the whole round in one ACT instruction
                        nce = len(chunks)
                        nc.scalar.activation(p_sb[:, :nce, sp:],
                                             s_ps[:, :nce, sp:], Exp)
                        # AV matmuls stream the exact allowed suffixes
                        for c, j in enumerate(chunks):
                            qs = qstarts[j]
                            if qs >= LQ:
                                continue
                            nc.tensor.matmul(
                                o_ps[:, qs:],
                                lhsT=v_sb[:, j * 128:(j + 1) * 128],
                                rhs=p_sb[:, c, qs:],
                                start=(j == 0), stop=(j == NCHUNK - 1))
                        # denominator: DVE-accumulate the round's P chunks
                        # (bf16, 2x mode), then one ones-matmul into PSUM.
                        live = [(qstarts[j], c) for c, j in enumerate(chunks)
                                if qstarts[j] < LQ]
                        if not live:
                            continue
                        live.sort()
                        qs0, c0 = live[0]
                        if len(live) == 1:
                            acc_ap = p_sb[:, c0, qs0:]
                        else:
                            acc = accpool.tile([128, LQ], bf16, tag="a")
                            qs1, c1 = live[1]
                            nc.vector.tensor_add(
                                acc[:, qs1:], p_sb[:, c0, qs1:], p_sb[:, c1, qs1:])
                            if qs1 > qs0:
                                nc.vector.tensor_copy(
                                    acc[:, qs0:qs1], p_sb[:, c0, qs0:qs1])
                            for qs2, c2 in live[2:]:
                                nc.vector.tensor_add(
                                    acc[:, qs2:], acc[:, qs2:], p_sb[:, c2, qs2:])
                            acc_ap = acc[:, qs0:]
                        nc.tensor.matmul(
                            d_ps[:, qs0:], lhsT=ones[:], rhs=acc_ap,
                            start=(r == 0), stop=(r == len(rounds) - 1))
                    o_sb = opool.tile([128, LQ], f32, tag="ot")
                    nc.vector.tensor_copy(o_sb[:], o_ps[:])
                    d_sb = opool.tile([1, LQ], f32, tag="dt")
                    nc.vector.tensor_copy(d_sb[:], d_ps[:])
                    nc.sync.dma_start(outT[h * 128:(h + 1) * 128, :], o_sb[:])
                    nc.sync.dma_start(den[h:h + 1, :], d_sb[:])
    nc.compile()
    return nc


def _get_nc(qstarts):
    nc = _nc_cache.get(qstarts)
    if nc is None:
        nc = _build_nc(qstarts)
        _nc_cache[qstarts] = nc
    return nc


def _core_inputs(c, q, k, v, k_cache, v_cache, block_tables):
    b, half = divmod(c, 2)
    kvh = slice(half * KV_PER_CORE, (half + 1) * KV_PER_CORE)
    qh = slice(half * H_PER_CORE, (half + 1) * H_PER_CORE)
    # paged gather + concat of current step, this core's kv heads: [LK, KV, D]
    Kc = np.concatenate([
        k_cache[block_tables[b]].reshape(CTX, HKV, D)[:, kvh],
        k[b][:, kvh]], axis=0)
    Vc = np.concatenate([
        v_cache[block_tables[b]].reshape(CTX, HKV, D)[:, kvh],
        v[b][:, kvh]], axis=0)
    # kT[g*128 + d, kk] = Kc[kk, g, d]
    kT = np.ascontiguousarray(Kc.transpose(1, 2, 0)).reshape(KV_PER_CORE * D, LK)
    # vT[g*128 + p, j*128 + d] = Vc[j*128 + p, g, d], bf16 on device
    import ml_dtypes
    vT = np.ascontiguousarray(
        Vc.reshape(NCHUNK, 128, KV_PER_CORE, D).transpose(2, 1, 0, 3)
    ).reshape(KV_PER_CORE * 128, NCHUNK * D).astype(ml_dtypes.bfloat16)
    # qT[h*128 + d, i] = q[b, i, qh][i, h, d] * SCALE
    qT = np.ascontiguousarray(
        (q[b][:, qh] * SCALE).transpose(1, 2, 0)
    ).reshape(H_PER_CORE * D, LQ)
    return {"qT": qT, "kT": kT, "vT": vT}


def _run(q, k, v, k_cache, v_cache, block_tables, allow_mask,
         trace=False, tmpdir=None):
    from concourse.bass_utils import run_bass_kernel_spmd

    q = np.asarray(q, dtype=np.float32)
    k = np.asarray(k, dtype=np.float32)
    v = np.asarray(v, dtype=np.float32)
    k_cache = np.asarray(k_cache, dtype=np.float32)
    v_cache = np.asarray(v_cache, dtype=np.float32)
    block_tables = np.asarray(block_tables)

    qstarts = _derive_qstarts(allow_mask)
    nc = _get_nc(qstarts)
    in_maps = [_core_inputs(c, q, k, v, k_cache, v_cache, block_tables)
               for c in range(N_CORES)]
    res = run_bass_kernel_spmd(nc, in_maps, core_ids=list(range(N_CORES)),
                               trace=trace, tmpdir=tmpdir)

    out = np.empty((B, LQ, HQ, D), dtype=np.float32)
    for c in range(N_CORES):
        b, half = divmod(c, 2)
        oT = np.asarray(res.results[c]["outT"]).reshape(H_PER_CORE, D, LQ)
        dn = np.asarray(res.results[c]["den"])          # [H_PER_CORE, LQ]
        o = oT / dn[:, None, :]
        out[b, :, half * H_PER_CORE:(half + 1) * H_PER_CORE, :] = \
            o.transpose(2, 0, 1)
    return out, res


def kernel(q, k, v, k_cache, v_cache, block_tables, allow_mask):
    out, _ = _run(q, k, v, k_cache, v_cache, block_tables, allow_mask)
    return out


# revision 12
# speedup vs baseline: 1.5927x; 1.0098x over previous
"""Paged block-attention (GQA, diffusion-block causal mask) on 8 Trainium2 cores.

Problem geometry (hardcoded; matches nn_BlockAttention_25778393710607):
  q       [B=4, LQ=512, HQ=16, D=128]
  k, v    [B=4, LQ=512, HKV=8, D=128]
  k_cache/v_cache [NUM_BLOCKS=64, BLOCK_SIZE=256, HKV=8, D=128]
  block_tables [B=4, BLOCKS_PER_SEQ=8] int32
  allow_mask [B=4, LQ=512, LK=2560] bool
  out     [B=4, LQ=512, HQ=16, D=128] fp32

Sharding: core c owns sequence c//2 and head-half c%2 (4 KV heads -> 8 Q
heads via GQA rep=2). The paged gather (cache rows per block table) plus
layout transposes happen on host while building each core's input map; the
device kernel computes, per (q-head):

  S^T[k, i] = (K_all @ (q*scale)^T)   chunk-wise over 20 key chunks of 128
  P = exp(S^T)                        (no max subtraction: |s| <~ 12 for
                                       randn inputs, fp32 exp is safe)
  outT[d, i] = sum_k V[k, d] * P[k, i]   (PSUM accumulation)
  den[i]    = sum_k P[k, i]              (ones-column matmul, PSUM accum)

and the host divides outT/den (softmax normalization) when reassembling.

The mask is applied structurally: for every 128-key chunk the set of
allowed queries is a suffix [qs, LQ) (true for the reference block-causal
mask with DIFF_BLOCK=128, and for an all-ones mask); only those query
columns are streamed through the PE for that chunk, so masked (k, q)
pairs are never computed and never pollute the denominator.
"""

import numpy as np

B, LQ, HQ, HKV, D = 4, 512, 16, 8, 128
BLOCK_SIZE, BLOCKS_PER_SEQ, NUM_BLOCKS = 256, 8, 64
CTX = BLOCK_SIZE * BLOCKS_PER_SEQ
LK = CTX + LQ
NCHUNK = LK // 128            # 20 key chunks of 128
SCALE = 1.0 / float(np.sqrt(D))
N_CORES = 8
H_PER_CORE = HQ // 2          # 8 q heads per core
KV_PER_CORE = HKV // 2        # 4 kv heads per core
CHUNKS_PER_ROUND = 2          # S^T chunks exp'd per ACT instruction

_nc_cache = {}


def _derive_qstarts(allow_mask):
    """Per key-chunk allowed-query suffix start, verified against the mask."""
    m = np.asarray(allow_mask, dtype=bool)
    assert m.shape == (B, LQ, LK), m.shape
    qstarts = []
    ar = np.arange(LQ)
    for j in range(NCHUNK):
        mj = m[:, :, j * 128:(j + 1) * 128]
        row = mj.any(axis=2)                      # [B, LQ]
        if not (mj == row[:, :, None]).all():
            raise ValueError(f"mask chunk {j} not uniform within the chunk")
        r0 = row[0]
        if not (row == r0[None]).all():
            raise ValueError(f"mask chunk {j} differs across batch")
        qs = int(LQ - r0.sum())
        if not (r0 == (ar >= qs)).all():
            raise ValueError(f"mask chunk {j} rows are not a query suffix")
        qstarts.append(qs)
    return tuple(qstarts)


def _build_nc(qstarts):
    import concourse.bass as bass
    import concourse.tile as tile
    from concourse import bacc, mybir

    f32 = mybir.dt.float32
    f32r = mybir.dt.float32r
    bf16 = mybir.dt.bfloat16
    Exp = mybir.ActivationFunctionType.Exp

    nc = bacc.Bacc("TRN2", target_bir_lowering=False, debug=False)
    qT = nc.dram_tensor("qT", [H_PER_CORE * 128, LQ], f32r, kind="ExternalInput").ap()
    kT = nc.dram_tensor("kT", [KV_PER_CORE * 128, LK], f32r, kind="ExternalInput").ap()
    vT = nc.dram_tensor("vT", [KV_PER_CORE * 128, LK], bf16, kind="ExternalInput").ap()
    outT = nc.dram_tensor("outT", [H_PER_CORE * 128, LQ], f32, kind="ExternalOutput").ap()
    den = nc.dram_tensor("den", [H_PER_CORE, LQ], f32, kind="ExternalOutput").ap()

    ROUND = 3                                      # key chunks per round
    rounds = [list(range(r, min(r + ROUND, NCHUNK)))
              for r in range(0, NCHUNK, ROUND)]
    # S^T/exp padded suffix start per round: uniform within the round so the
    # round's exp is one (possibly 3D-AP) ACT instruction, and capped at 256
    # so fp32r matmuls keep their N>=256 full-rate mode. AV/denominator use
    # the exact per-chunk suffix, so padded (masked) scores are never used.
    spad = [min(min(qstarts[j] for j in ch), LQ - 256) for ch in rounds]
    assert qstarts[0] == 0, "first key chunk must be unmasked"

    with tile.TileContext(nc) as tc:
        with tc.tile_pool(name="const", bufs=1) as cpool, \
             tc.tile_pool(name="qpool", bufs=1) as qpool, \
             tc.tile_pool(name="kv", bufs=2) as kvpool, \
             tc.tile_pool(name="pp", bufs=3) as ppool, \
             tc.tile_pool(name="acc", bufs=2) as accpool, \
             tc.tile_pool(name="ostage", bufs=2) as opool, \
             tc.tile_pool(name="psum", bufs=2, space="PSUM") as pspool:

            ones = cpool.tile([128, 1], bf16)
            nc.vector.memset(ones[:], 1.0)
            warm = cpool.tile([128, LQ], bf16)
            nc.vector.memset(warm[:], 0.0)

            q_sb = qpool.tile([128, H_PER_CORE, LQ], f32r)

            for g in range(KV_PER_CORE):
                k_sb = kvpool.tile([128, LK], f32r, tag="k")
                nc.sync.dma_start(k_sb[:, :LK // 2], kT[g * 128:(g + 1) * 128, :LK // 2])
                nc.sync.dma_start(k_sb[:, LK // 2:], kT[g * 128:(g + 1) * 128, LK // 2:])
                v_sb = kvpool.tile([128, LK], bf16, tag="v")
                nc.sync.dma_start(v_sb[:, :LK // 2], vT[g * 128:(g + 1) * 128, :LK // 2])
                nc.sync.dma_start(v_sb[:, LK // 2:], vT[g * 128:(g + 1) * 128, LK // 2:])

                if g == 0:
                    # prefetch every q head now (sync FIFO: after k0/v0)
                    for h in range(H_PER_CORE):
                        nc.sync.dma_start(q_sb[:, h, :],
                                          qT[h * 128:(h + 1) * 128, :])
                    # keep the PE busy through the DMA prologue so the HAM
                    # clock gate is warm (2.4 GHz) when real matmuls start
                    wps = pspool.tile([1, LQ], f32, tag="d", bufs=1)
                    for _ in range(32):
                        nc.tensor.matmul(wps[:], lhsT=ones[:], rhs=warm[:],
                                         start=True, stop=True)

                for h2 in range(2):
                    h = 2 * g + h2
                    o_ps = pspool.tile([128, LQ], f32, tag="o", bufs=1)
                    d_ps = pspool.tile([1, LQ], f32, tag="d", bufs=1)
                    for r, chunks in enumerate(rounds):
                        sp = spad[r]
                        width = LQ - sp
                        s_ps = pspool.tile([128, ROUND, LQ], f32, tag="s")
                        p_sb = ppool.tile([128, ROUND, LQ], bf16, tag="p")
                        for c, j in enumerate(chunks):
                            nc.tensor.matmul(
                                s_ps[:, c, sp:],
                                lhsT=k_sb[:, j * 128:(j + 1) * 128],
                                rhs=q_sb[:, h, sp:],
                                start=True, stop=True)
                        # exp of the whole round in one ACT instruction
                        nce = len(chunks)
                        nc.scalar.activation(p_sb[:, :nce, sp:],
                                             s_ps[:, :nce, sp:], Exp)
                        # AV matmuls stream the exact allowed suffixes
                        for c, j in enumerate(chunks):
                            qs = qstarts[j]
                            if qs >= LQ:
                                continue
                            nc.tensor.matmul(
                                o_ps[:, qs:],
                                lhsT=v_sb[:, j * 128:(j + 1) * 128],
                                rhs=p_sb[:, c, qs:],
                                start=(j == 0), stop=(j == NCHUNK - 1))
                        # denominator: DVE-accumulate the round's P chunks
                        # (bf16, 2x mode), then one ones-matmul into PSUM.
                        live = [(qstarts[j], c) for c, j in enumerate(chunks)
                                if qstarts[j] < LQ]
                        if not live:
                            continue
                        live.sort()
                        qs0, c0 = live[0]
                        if len(live) == 1:
                            acc_ap = p_sb[:, c0, qs0:]
                        else:
                            acc = accpool.tile([128, LQ], bf16, tag="a")
                            qs1, c1 = live[1]
                            nc.vector.tensor_add(
                                acc[:, qs1:], p_sb[:, c0, qs1:], p_sb[:, c1, qs1:])
                            if qs1 > qs0:
                                nc.vector.tensor_copy(
                                    acc[:, qs0:qs1], p_sb[:, c0, qs0:qs1])
                            for qs2, c2 in live[2:]:
                                nc.vector.tensor_add(
                                    acc[:, qs2:], acc[:, qs2:], p_sb[:, c2, qs2:])
                            acc_ap = acc[:, qs0:]
                        nc.tensor.matmul(
                            d_ps[:, qs0:], lhsT=ones[:], rhs=acc_ap,
                            start=(r == 0), stop=(r == len(rounds) - 1))
                    o_sb = opool.tile([128, LQ], f32, tag="ot")
                    nc.vector.tensor_copy(o_sb[:], o_ps[:])
                    d_sb = opool.tile([1, LQ], f32, tag="dt")
                    nc.vector.tensor_copy(d_sb[:], d_ps[:])
                    nc.sync.dma_start(outT[h * 128:(h + 1) * 128, :], o_sb[:])
                    nc.sync.dma_start(den[h:h + 1, :], d_sb[:])
    nc.compile()
    return nc


def _get_nc(qstarts):
    nc = _nc_cache.get(qstarts)
    if nc is None:
        nc = _build_nc(qstarts)
        _nc_cache[qstarts] = nc
    return nc


def _core_inputs(c, q, k, v, k_cache, v_cache, block_tables):
    b, half = divmod(c, 2)
    kvh = slice(half * KV_PER_CORE, (half + 1) * KV_PER_CORE)
    qh = slice(half * H_PER_CORE, (half + 1) * H_PER_CORE)
    # paged gather + concat of current step, this core's kv heads: [LK, KV, D]
    Kc = np.concatenate([
        k_cache[block_tables[b]].reshape(CTX, HKV, D)[:, kvh],
        k[b][:, kvh]], axis=0)
    Vc = np.concatenate([
        v_cache[block_tables[b]].reshape(CTX, HKV, D)[:, kvh],
        v[b][:, kvh]], axis=0)
    # kT[g*128 + d, kk] = Kc[kk, g, d]
    kT = np.ascontiguousarray(Kc.transpose(1, 2, 0)).reshape(KV_PER_CORE * D, LK)
    # vT[g*128 + p, j*128 + d] = Vc[j*128 + p, g, d], bf16 on device
    import ml_dtypes
    vT = np.ascontiguousarray(
        Vc.reshape(NCHUNK, 128, KV_PER_CORE, D).transpose(2, 1, 0, 3)
    ).reshape(KV_PER_CORE * 128, NCHUNK * D).astype(ml_dtypes.bfloat16)
    # qT[h*128 + d, i] = q[b, i, qh][i, h, d] * SCALE
    qT = np.ascontiguousarray(
        (q[b][:, qh] * SCALE).transpose(1, 2, 0)
    ).reshape(H_PER_CORE * D, LQ)
    return {"qT": qT, "kT": kT, "vT": vT}


def _run(q, k, v, k_cache, v_cache, block_tables, allow_mask,
         trace=False, tmpdir=None):
    from concourse.bass_utils import run_bass_kernel_spmd

    q = np.asarray(q, dtype=np.float32)
    k = np.asarray(k, dtype=np.float32)
    v = np.asarray(v, dtype=np.float32)
    k_cache = np.asarray(k_cache, dtype=np.float32)
    v_cache = np.asarray(v_cache, dtype=np.float32)
    block_tables = np.asarray(block_tables)

    qstarts = _derive_qstarts(allow_mask)
    nc = _get_nc(qstarts)
    in_maps = [_core_inputs(c, q, k, v, k_cache, v_cache, block_tables)
               for c in range(N_CORES)]
    res = run_bass_kernel_spmd(nc, in_maps, core_ids=list(range(N_CORES)),
                               trace=trace, tmpdir=tmpdir)

    out = np.empty((B, LQ, HQ, D), dtype=np.float32)
    for c in range(N_CORES):
        b, half = divmod(c, 2)
        oT = np.asarray(res.results[c]["outT"]).reshape(H_PER_CORE, D, LQ)
        dn = np.asarray(res.results[c]["den"])          # [H_PER_CORE, LQ]
        o = oT / dn[:, None, :]
        out[b, :, half * H_PER_CORE:(half + 1) * H_PER_CORE, :] = \
            o.transpose(2, 0, 1)
    return out, res


def kernel(q, k, v, k_cache, v_cache, block_tables, allow_mask):
    out, _ = _run(q, k, v, k_cache, v_cache, block_tables, allow_mask)
    return out


# revision 14
# speedup vs baseline: 1.6258x; 1.0207x over previous
"""Paged block-attention (GQA, diffusion-block causal mask) on 8 Trainium2 cores.

Problem geometry (hardcoded; matches nn_BlockAttention_25778393710607):
  q       [B=4, LQ=512, HQ=16, D=128]
  k, v    [B=4, LQ=512, HKV=8, D=128]
  k_cache/v_cache [NUM_BLOCKS=64, BLOCK_SIZE=256, HKV=8, D=128]
  block_tables [B=4, BLOCKS_PER_SEQ=8] int32
  allow_mask [B=4, LQ=512, LK=2560] bool
  out     [B=4, LQ=512, HQ=16, D=128] fp32

Sharding: core c owns sequence c//2 and head-half c%2 (4 KV heads -> 8 Q
heads via GQA rep=2). The paged gather (cache rows per block table) plus
layout transposes happen on host while building each core's input map; the
device kernel computes, per (q-head):

  S^T[k, i] = (K_all @ (q*scale)^T)   chunk-wise over 20 key chunks of 128
  P = exp(S^T)                        (no max subtraction: |s| <~ 12 for
                                       randn inputs, fp32 exp is safe)
  outT[d, i] = sum_k V[k, d] * P[k, i]   (PSUM accumulation)
  den[i]    = sum_k P[k, i]              (ones-column matmul, PSUM accum)

and the host divides outT/den (softmax normalization) when reassembling.

The mask is applied structurally: for every 128-key chunk the set of
allowed queries is a suffix [qs, LQ) (true for the reference block-causal
mask with DIFF_BLOCK=128, and for an all-ones mask); only those query
columns are streamed through the PE for that chunk, so masked (k, q)
pairs are never computed and never pollute the denominator.
"""

import numpy as np

B, LQ, HQ, HKV, D = 4, 512, 16, 8, 128
BLOCK_SIZE, BLOCKS_PER_SEQ, NUM_BLOCKS = 256, 8, 64
CTX = BLOCK_SIZE * BLOCKS_PER_SEQ
LK = CTX + LQ
NCHUNK = LK // 128            # 20 key chunks of 128
SCALE = 1.0 / float(np.sqrt(D))
N_CORES = 8
H_PER_CORE = HQ // 2          # 8 q heads per core
KV_PER_CORE = HKV // 2        # 4 kv heads per core
CHUNKS_PER_ROUND = 2          # S^T chunks exp'd per ACT instruction

_nc_cache = {}


def _derive_qstarts(allow_mask):
    """Per key-chunk allowed-query suffix start, verified against the mask."""
    m = np.asarray(allow_mask, dtype=bool)
    assert m.shape == (B, LQ, LK), m.shape
    qstarts = []
    ar = np.arange(LQ)
    for j in range(NCHUNK):
        mj = m[:, :, j * 128:(j + 1) * 128]
        row = mj.any(axis=2)                      # [B, LQ]
        if not (mj == row[:, :, None]).all():
            raise ValueError(f"mask chunk {j} not uniform within the chunk")
        r0 = row[0]
        if not (row == r0[None]).all():
            raise ValueError(f"mask chunk {j} differs across batch")
        qs = int(LQ - r0.sum())
        if not (r0 == (ar >= qs)).all():
            raise ValueError(f"mask chunk {j} rows are not a query suffix")
        qstarts.append(qs)
    return tuple(qstarts)


def _build_nc(qstarts):
    import concourse.bass as bass
    import concourse.tile as tile
    from concourse import bacc, mybir

    f32 = mybir.dt.float32
    f32r = mybir.dt.float32r
    bf16 = mybir.dt.bfloat16
    Exp = mybir.ActivationFunctionType.Exp

    nc = bacc.Bacc("TRN2", target_bir_lowering=False, debug=False)
    qT = nc.dram_tensor("qT", [H_PER_CORE * 128, LQ], f32r, kind="ExternalInput").ap()
    kT = nc.dram_tensor("kT", [KV_PER_CORE * 128, LK], f32r, kind="ExternalInput").ap()
    vT = nc.dram_tensor("vT", [KV_PER_CORE * 128, LK], bf16, kind="ExternalInput").ap()
    outT = nc.dram_tensor("outT", [H_PER_CORE * 128, LQ], f32, kind="ExternalOutput").ap()
    den = nc.dram_tensor("den", [H_PER_CORE, LQ], f32, kind="ExternalOutput").ap()

    ROUND = 3                                      # key chunks per round
    rounds = [list(range(r, min(r + ROUND, NCHUNK)))
              for r in range(0, NCHUNK, ROUND)]
    # S^T/exp padded suffix start per round: uniform within the round so the
    # round's exp is one (possibly 3D-AP) ACT instruction, and capped at 256
    # so fp32r matmuls keep their N>=256 full-rate mode. AV/denominator use
    # the exact per-chunk suffix, so padded (masked) scores are never used.
    spad = [min(min(qstarts[j] for j in ch), LQ - 256) for ch in rounds]
    assert qstarts[0] == 0, "first key chunk must be unmasked"

    with tile.TileContext(nc) as tc:
        with tc.tile_pool(name="const", bufs=1) as cpool, \
             tc.tile_pool(name="qpool", bufs=1) as qpool, \
             tc.tile_pool(name="kv", bufs=2) as kvpool, \
             tc.tile_pool(name="pp", bufs=3) as ppool, \
             tc.tile_pool(name="acc", bufs=2) as accpool, \
             tc.tile_pool(name="ostage", bufs=2) as opool, \
             tc.tile_pool(name="psum", bufs=2, space="PSUM") as pspool:

            ones = cpool.tile([128, 1], bf16)
            nc.vector.memset(ones[:], 1.0)
            warm = cpool.tile([128, LQ], bf16)
            nc.vector.memset(warm[:], 0.0)

            q_sb = qpool.tile([128, H_PER_CORE, LQ], f32r)

            n_pairs = KV_PER_CORE * 2
            kv_tiles = [None] * KV_PER_CORE     # g -> (k_sb, v_sb)
            state = {}                          # h -> per-pair psum/stage

            def load_kv(g):
                k_sb = kvpool.tile([128, LK], f32r, tag="k")
                nc.sync.dma_start(k_sb[:, :LK // 2],
                                  kT[g * 128:(g + 1) * 128, :LK // 2])
                nc.sync.dma_start(k_sb[:, LK // 2:],
                                  kT[g * 128:(g + 1) * 128, LK // 2:])
                v_sb = kvpool.tile([128, LK], bf16, tag="v")
                nc.sync.dma_start(v_sb[:, :LK // 2],
                                  vT[g * 128:(g + 1) * 128, :LK // 2])
                nc.sync.dma_start(v_sb[:, LK // 2:],
                                  vT[g * 128:(g + 1) * 128, LK // 2:])
                kv_tiles[g] = (k_sb, v_sb)

            def emit_front(h, r):
                # S^T matmuls + exp for (pair h, round r)
                g = h // 2
                k_sb, _ = kv_tiles[g]
                chunks = rounds[r]
                sp = spad[r]
                s_ps = pspool.tile([128, ROUND, LQ], f32, tag="s")
                p_sb = ppool.tile([128, ROUND, LQ], bf16, tag="p")
                for c, j in enumerate(chunks):
                    nc.tensor.matmul(
                        s_ps[:, c, sp:],
                        lhsT=k_sb[:, j * 128:(j + 1) * 128],
                        rhs=q_sb[:, h, sp:],
                        start=True, stop=True)
                nce = len(chunks)
                nc.scalar.activation(p_sb[:, :nce, sp:], s_ps[:, :nce, sp:], Exp)
                return p_sb

            def emit_back(h, r, p_sb):
                # AV + denominator for (pair h, round r), plus pair drain
                g = h // 2
                _, v_sb = kv_tiles[g]
                chunks = rounds[r]
                if r == 0:
                    state[h] = (
                        pspool.tile([128, LQ], f32, tag="o", bufs=1,
                                    name=f"o_ps_{h}"),
                        pspool.tile([1, LQ], f32, tag="d", bufs=1,
                                    name=f"d_ps_{h}"))
                o_ps, d_ps = state[h]
                for c, j in enumerate(chunks):
                    qs = qstarts[j]
                    if qs >= LQ:
                        continue
                    nc.tensor.matmul(
                        o_ps[:, qs:],
                        lhsT=v_sb[:, j * 128:(j + 1) * 128],
                        rhs=p_sb[:, c, qs:],
                        start=(j == 0), stop=(j == NCHUNK - 1))
                live = sorted((qstarts[j], c) for c, j in enumerate(chunks)
                              if qstarts[j] < LQ)
                if live:
                    qs0, c0 = live[0]
                    if len(live) == 1:
                        acc_ap = p_sb[:, c0, qs0:]
                    else:
                        acc = accpool.tile([128, LQ], bf16, tag="a")
                        qs1, c1 = live[1]
                        nc.vector.tensor_add(
                            acc[:, qs1:], p_sb[:, c0, qs1:], p_sb[:, c1, qs1:])
                        if qs1 > qs0:
                            nc.vector.tensor_copy(
                                acc[:, qs0:qs1], p_sb[:, c0, qs0:qs1])
                        for qs2, c2 in live[2:]:
                            nc.vector.tensor_add(
                                acc[:, qs2:], acc[:, qs2:], p_sb[:, c2, qs2:])
                        acc_ap = acc[:, qs0:]
                    nc.tensor.matmul(
                        d_ps[:, qs0:], lhsT=ones[:], rhs=acc_ap,
                        start=(r == 0), stop=(r == len(rounds) - 1))
                if r == len(rounds) - 1:
                    o_sb = opool.tile([128, LQ], f32, tag="ot")
                    nc.vector.tensor_copy(o_sb[:], o_ps[:])
                    d_sb = opool.tile([1, LQ], f32, tag="dt")
                    nc.vector.tensor_copy(d_sb[:], d_ps[:])
                    nc.sync.dma_start(outT[h * 128:(h + 1) * 128, :], o_sb[:])
                    nc.sync.dma_start(den[h:h + 1, :], d_sb[:])
                    del state[h]

            # prologue: first kv head, q0 early, then the other q heads,
            # and PE-warmup matmuls to lift the HAM clock gate before the
            # first real matmul issues
            load_kv(0)
            nc.sync.dma_start(q_sb[:, 0, :], qT[0:128, :])
            for h in range(1, H_PER_CORE):
                nc.sync.dma_start(q_sb[:, h, :], qT[h * 128:(h + 1) * 128, :])
            wps = pspool.tile([1, LQ], f32, tag="d", bufs=1)
            for _ in range(10):
                nc.tensor.matmul(wps[:], lhsT=ones[:], rhs=warm[:],
                                 start=True, stop=True)

            # one-round software pipeline across the flat task stream so
            # ACT never sits behind the previous pair's PE tail
            tasks = [(h, r) for h in range(n_pairs) for r in range(len(rounds))]
            prev = None
            for h, r in tasks:
                if r == 0 and h % 2 == 0 and h // 2 + 1 < KV_PER_CORE:
                    load_kv(h // 2 + 1)
                p_sb = emit_front(h, r)
                if prev is not None:
                    emit_back(prev[0], prev[1], prev[2])
                prev = (h, r, p_sb)
            emit_back(prev[0], prev[1], prev[2])
    nc.compile()
    return nc


def _get_nc(qstarts):
    nc = _nc_cache.get(qstarts)
    if nc is None:
        nc = _build_nc(qstarts)
        _nc_cache[qstarts] = nc
    return nc


def _core_inputs(c, q, k, v, k_cache, v_cache, block_tables):
    b, half = divmod(c, 2)
    kvh = slice(half * KV_PER_CORE, (half + 1) * KV_PER_CORE)
    qh = slice(half * H_PER_CORE, (half + 1) * H_PER_CORE)
    # paged gather + concat of current step, this core's kv heads: [LK, KV, D]
    Kc = np.concatenate([
        k_cache[block_tables[b]].reshape(CTX, HKV, D)[:, kvh],
        k[b][:, kvh]], axis=0)
    Vc = np.concatenate([
        v_cache[block_tables[b]].reshape(CTX, HKV, D)[:, kvh],
        v[b][:, kvh]], axis=0)
    # kT[g*128 + d, kk] = Kc[kk, g, d]
    kT = np.ascontiguousarray(Kc.transpose(1, 2, 0)).reshape(KV_PER_CORE * D, LK)
    # vT[g*128 + p, j*128 + d] = Vc[j*128 + p, g, d], bf16 on device
    import ml_dtypes
    vT = np.ascontiguousarray(
        Vc.reshape(NCHUNK, 128, KV_PER_CORE, D).transpose(2, 1, 0, 3)
    ).reshape(KV_PER_CORE * 128, NCHUNK * D).astype(ml_dtypes.bfloat16)
    # qT[h*128 + d, i] = q[b, i, qh][i, h, d] * SCALE
    qT = np.ascontiguousarray(
        (q[b][:, qh] * SCALE).transpose(1, 2, 0)
    ).reshape(H_PER_CORE * D, LQ)
    return {"qT": qT, "kT": kT, "vT": vT}


def _run(q, k, v, k_cache, v_cache, block_tables, allow_mask,
         trace=False, tmpdir=None):
    from concourse.bass_utils import run_bass_kernel_spmd

    q = np.asarray(q, dtype=np.float32)
    k = np.asarray(k, dtype=np.float32)
    v = np.asarray(v, dtype=np.float32)
    k_cache = np.asarray(k_cache, dtype=np.float32)
    v_cache = np.asarray(v_cache, dtype=np.float32)
    block_tables = np.asarray(block_tables)

    qstarts = _derive_qstarts(allow_mask)
    nc = _get_nc(qstarts)
    in_maps = [_core_inputs(c, q, k, v, k_cache, v_cache, block_tables)
               for c in range(N_CORES)]
    res = run_bass_kernel_spmd(nc, in_maps, core_ids=list(range(N_CORES)),
                               trace=trace, tmpdir=tmpdir)

    out = np.empty((B, LQ, HQ, D), dtype=np.float32)
    for c in range(N_CORES):
        b, half = divmod(c, 2)
        oT = np.asarray(res.results[c]["outT"]).reshape(H_PER_CORE, D, LQ)
        dn = np.asarray(res.results[c]["den"])          # [H_PER_CORE, LQ]
        o = oT / dn[:, None, :]
        out[b, :, half * H_PER_CORE:(half + 1) * H_PER_CORE, :] = \
            o.transpose(2, 0, 1)
    return out, res


def kernel(q, k, v, k_cache, v_cache, block_tables, allow_mask):
    out, _ = _run(q, k, v, k_cache, v_cache, block_tables, allow_mask)
    return out


# revision 16
# speedup vs baseline: 1.6967x; 1.0436x over previous
"""Paged block-attention (GQA, diffusion-block causal mask) on 8 Trainium2 cores.

Problem geometry (hardcoded; matches nn_BlockAttention_25778393710607):
  q       [B=4, LQ=512, HQ=16, D=128]
  k, v    [B=4, LQ=512, HKV=8, D=128]
  k_cache/v_cache [NUM_BLOCKS=64, BLOCK_SIZE=256, HKV=8, D=128]
  block_tables [B=4, BLOCKS_PER_SEQ=8] int32
  allow_mask [B=4, LQ=512, LK=2560] bool
  out     [B=4, LQ=512, HQ=16, D=128] fp32

Sharding: core c owns sequence c//2 and head-half c%2 (4 KV heads -> 8 Q
heads via GQA rep=2). The paged gather (cache rows per block table) plus
layout transposes happen on host while building each core's input map; the
device kernel computes, per (q-head):

  S^T[k, i] = (K_all @ (q*scale)^T)   chunk-wise over 20 key chunks of 128
  P = exp(S^T)                        (no max subtraction: |s| <~ 12 for
                                       randn inputs, fp32 exp is safe)
  outT[d, i] = sum_k V[k, d] * P[k, i]   (PSUM accumulation)
  den[i]    = sum_k P[k, i]              (ones-column matmul, PSUM accum)

and the host divides outT/den (softmax normalization) when reassembling.

The mask is applied structurally: for every 128-key chunk the set of
allowed queries is a suffix [qs, LQ) (true for the reference block-causal
mask with DIFF_BLOCK=128, and for an all-ones mask); only those query
columns are streamed through the PE for that chunk, so masked (k, q)
pairs are never computed and never pollute the denominator.
"""

import numpy as np

B, LQ, HQ, HKV, D = 4, 512, 16, 8, 128
BLOCK_SIZE, BLOCKS_PER_SEQ, NUM_BLOCKS = 256, 8, 64
CTX = BLOCK_SIZE * BLOCKS_PER_SEQ
LK = CTX + LQ
NCHUNK = LK // 128            # 20 key chunks of 128
SCALE = 1.0 / float(np.sqrt(D))
N_CORES = 8
H_PER_CORE = HQ // 2          # 8 q heads per core
KV_PER_CORE = HKV // 2        # 4 kv heads per core
CHUNKS_PER_ROUND = 2          # S^T chunks exp'd per ACT instruction

_nc_cache = {}


def _derive_qstarts(allow_mask):
    """Per key-chunk allowed-query suffix start, verified against the mask."""
    m = np.asarray(allow_mask, dtype=bool)
    assert m.shape == (B, LQ, LK), m.shape
    qstarts = []
    ar = np.arange(LQ)
    for j in range(NCHUNK):
        mj = m[:, :, j * 128:(j + 1) * 128]
        row = mj.any(axis=2)                      # [B, LQ]
        if not (mj == row[:, :, None]).all():
            raise ValueError(f"mask chunk {j} not uniform within the chunk")
        r0 = row[0]
        if not (row == r0[None]).all():
            raise ValueError(f"mask chunk {j} differs across batch")
        qs = int(LQ - r0.sum())
        if not (r0 == (ar >= qs)).all():
            raise ValueError(f"mask chunk {j} rows are not a query suffix")
        qstarts.append(qs)
    return tuple(qstarts)


def _build_nc(qstarts):
    import concourse.bass as bass
    import concourse.tile as tile
    from concourse import bacc, mybir

    f32 = mybir.dt.float32
    f32r = mybir.dt.float32r
    bf16 = mybir.dt.bfloat16
    Exp = mybir.ActivationFunctionType.Exp

    nc = bacc.Bacc("TRN2", target_bir_lowering=False, debug=False)
    qT = nc.dram_tensor("qT", [H_PER_CORE * 128, LQ], f32r, kind="ExternalInput").ap()
    kT = nc.dram_tensor("kT", [KV_PER_CORE * 128, LK], f32r, kind="ExternalInput").ap()
    vT = nc.dram_tensor("vT", [KV_PER_CORE * 128, LK], bf16, kind="ExternalInput").ap()
    outT = nc.dram_tensor("outT", [H_PER_CORE * 128, LQ], f32, kind="ExternalOutput").ap()
    den = nc.dram_tensor("den", [H_PER_CORE, LQ], f32, kind="ExternalOutput").ap()

    ROUND = 3                                      # key chunks per round
    rounds = [list(range(r, min(r + ROUND, NCHUNK)))
              for r in range(0, NCHUNK, ROUND)]
    # S^T/exp padded suffix start per round: uniform within the round so the
    # round's exp is one (possibly 3D-AP) ACT instruction, and capped at 256
    # so fp32r matmuls keep their N>=256 full-rate mode. AV/denominator use
    # the exact per-chunk suffix, so padded (masked) scores are never used.
    spad = [min(min(qstarts[j] for j in ch), LQ - 256) for ch in rounds]
    assert qstarts[0] == 0, "first key chunk must be unmasked"

    with tile.TileContext(nc) as tc:
        with tc.tile_pool(name="const", bufs=1) as cpool, \
             tc.tile_pool(name="qpool", bufs=1) as qpool, \
             tc.tile_pool(name="kv", bufs=2) as kvpool, \
             tc.tile_pool(name="pp", bufs=4) as ppool, \
             tc.tile_pool(name="acc", bufs=2) as accpool, \
             tc.tile_pool(name="ostage", bufs=2) as opool, \
             tc.tile_pool(name="psum", bufs=2, space="PSUM") as pspool:

            ones = cpool.tile([128, 1], bf16)
            nc.vector.memset(ones[:], 1.0)
            warm = cpool.tile([128, LQ], bf16)
            nc.vector.memset(warm[:], 0.0)

            q_sb = qpool.tile([128, H_PER_CORE, LQ], f32r)

            n_pairs = KV_PER_CORE * 2
            kv_tiles = [None] * KV_PER_CORE     # g -> (k_sb, v_sb)
            state = {}                          # h -> per-pair psum/stage

            def load_kv(g):
                k_sb = kvpool.tile([128, LK], f32r, tag="k")
                nc.sync.dma_start(k_sb[:, :LK // 2],
                                  kT[g * 128:(g + 1) * 128, :LK // 2])
                nc.sync.dma_start(k_sb[:, LK // 2:],
                                  kT[g * 128:(g + 1) * 128, LK // 2:])
                v_sb = kvpool.tile([128, LK], bf16, tag="v")
                nc.sync.dma_start(v_sb[:, :LK // 2],
                                  vT[g * 128:(g + 1) * 128, :LK // 2])
                nc.sync.dma_start(v_sb[:, LK // 2:],
                                  vT[g * 128:(g + 1) * 128, LK // 2:])
                kv_tiles[g] = (k_sb, v_sb)

            def emit_front(h, r):
                # S^T matmuls + exp for (pair h, round r)
                g = h // 2
                k_sb, _ = kv_tiles[g]
                chunks = rounds[r]
                sp = spad[r]
                s_ps = pspool.tile([128, ROUND, LQ], f32, tag="s")
                p_sb = ppool.tile([128, ROUND, LQ], bf16, tag="p")
                for c, j in enumerate(chunks):
                    nc.tensor.matmul(
                        s_ps[:, c, sp:],
                        lhsT=k_sb[:, j * 128:(j + 1) * 128],
                        rhs=q_sb[:, h, sp:],
                        start=True, stop=True)
                nce = len(chunks)
                nc.scalar.activation(p_sb[:, :nce, sp:], s_ps[:, :nce, sp:], Exp)
                return p_sb

            def emit_back(h, r, p_sb):
                # AV + denominator for (pair h, round r), plus pair drain
                g = h // 2
                _, v_sb = kv_tiles[g]
                chunks = rounds[r]
                if r == 0:
                    state[h] = (
                        pspool.tile([128, LQ], f32, tag="o", bufs=1,
                                    name=f"o_ps_{h}"),
                        pspool.tile([1, LQ], f32, tag="d", bufs=1,
                                    name=f"d_ps_{h}"))
                o_ps, d_ps = state[h]
                for c, j in enumerate(chunks):
                    qs = qstarts[j]
                    if qs >= LQ:
                        continue
                    nc.tensor.matmul(
                        o_ps[:, qs:],
                        lhsT=v_sb[:, j * 128:(j + 1) * 128],
                        rhs=p_sb[:, c, qs:],
                        start=(j == 0), stop=(j == NCHUNK - 1))
                live = sorted((qstarts[j], c) for c, j in enumerate(chunks)
                              if qstarts[j] < LQ)
                if live:
                    qs0, c0 = live[0]
                    if len(live) == 1:
                        acc_ap = p_sb[:, c0, qs0:]
                    else:
                        acc = accpool.tile([128, LQ], bf16, tag="a")
                        qs1, c1 = live[1]
                        nc.vector.tensor_add(
                            acc[:, qs1:], p_sb[:, c0, qs1:], p_sb[:, c1, qs1:])
                        if qs1 > qs0:
                            nc.vector.tensor_copy(
                                acc[:, qs0:qs1], p_sb[:, c0, qs0:qs1])
                        for qs2, c2 in live[2:]:
                            nc.vector.tensor_add(
                                acc[:, qs2:], acc[:, qs2:], p_sb[:, c2, qs2:])
                        acc_ap = acc[:, qs0:]
                    nc.tensor.matmul(
                        d_ps[:, qs0:], lhsT=ones[:], rhs=acc_ap,
                        start=(r == 0), stop=(r == len(rounds) - 1))
                if r == len(rounds) - 1:
                    o_sb = opool.tile([128, LQ], f32, tag="ot")
                    nc.vector.tensor_copy(o_sb[:], o_ps[:])
                    d_sb = opool.tile([1, LQ], f32, tag="dt")
                    nc.vector.tensor_copy(d_sb[:], d_ps[:])
                    nc.sync.dma_start(outT[h * 128:(h + 1) * 128, :], o_sb[:])
                    nc.sync.dma_start(den[h:h + 1, :], d_sb[:])
                    del state[h]

            # prologue: q0 + the first k pieces on the sync ring (ordered so
            # round 0 unblocks earliest), v0 + late q heads on the gpsimd
            # (SWDGE) ring so the transfers overlap, and PE-warmup matmuls
            # to lift the HAM clock gate before the first real matmul
            k_sb0 = kvpool.tile([128, LK], f32r, tag="k")
            v_sb0 = kvpool.tile([128, LK], bf16, tag="v")
            kv_tiles[0] = (k_sb0, v_sb0)
            nc.sync.dma_start(q_sb[:, 0, :], qT[0:128, :])
            nc.sync.dma_start(k_sb0[:, :384], kT[0:128, :384])
            nc.sync.dma_start(k_sb0[:, 384:1280], kT[0:128, 384:1280])
            nc.sync.dma_start(k_sb0[:, 1280:], kT[0:128, 1280:])
            for h in range(1, 4):
                nc.sync.dma_start(q_sb[:, h, :], qT[h * 128:(h + 1) * 128, :])
            nc.gpsimd.dma_start(v_sb0[:, :LK // 2], vT[0:128, :LK // 2])
            nc.gpsimd.dma_start(v_sb0[:, LK // 2:], vT[0:128, LK // 2:])
            for h in range(4, H_PER_CORE):
                nc.gpsimd.dma_start(q_sb[:, h, :], qT[h * 128:(h + 1) * 128, :])
            wps = pspool.tile([1, LQ], f32, tag="d", bufs=1)
            for _ in range(10):
                nc.tensor.matmul(wps[:], lhsT=ones[:], rhs=warm[:],
                                 start=True, stop=True)

            # two-round software pipeline across the flat task stream so
            # neither PE nor ACT ever waits at the dependency frontier
            tasks = [(h, r) for h in range(n_pairs) for r in range(len(rounds))]
            pend = []
            for h, r in tasks:
                if r == 0 and h % 2 == 0 and h // 2 + 1 < KV_PER_CORE:
                    load_kv(h // 2 + 1)
                p_sb = emit_front(h, r)
                pend.append((h, r, p_sb))
                if len(pend) > 2:
                    emit_back(*pend.pop(0))
            for t in pend:
                emit_back(*t)
    nc.compile()
    return nc


def _get_nc(qstarts):
    nc = _nc_cache.get(qstarts)
    if nc is None:
        nc = _build_nc(qstarts)
        _nc_cache[qstarts] = nc
    return nc


def _core_inputs(c, q, k, v, k_cache, v_cache, block_tables):
    b, half = divmod(c, 2)
    kvh = slice(half * KV_PER_CORE, (half + 1) * KV_PER_CORE)
    qh = slice(half * H_PER_CORE, (half + 1) * H_PER_CORE)
    # paged gather + concat of current step, this core's kv heads: [LK, KV, D]
    Kc = np.concatenate([
        k_cache[block_tables[b]].reshape(CTX, HKV, D)[:, kvh],
        k[b][:, kvh]], axis=0)
    Vc = np.concatenate([
        v_cache[block_tables[b]].reshape(CTX, HKV, D)[:, kvh],
        v[b][:, kvh]], axis=0)
    # kT[g*128 + d, kk] = Kc[kk, g, d]
    kT = np.ascontiguousarray(Kc.transpose(1, 2, 0)).reshape(KV_PER_CORE * D, LK)
    # vT[g*128 + p, j*128 + d] = Vc[j*128 + p, g, d], bf16 on device
    import ml_dtypes
    vT = np.ascontiguousarray(
        Vc.reshape(NCHUNK, 128, KV_PER_CORE, D).transpose(2, 1, 0, 3)
    ).reshape(KV_PER_CORE * 128, NCHUNK * D).astype(ml_dtypes.bfloat16)
    # qT[h*128 + d, i] = q[b, i, qh][i, h, d] * SCALE
    qT = np.ascontiguousarray(
        (q[b][:, qh] * SCALE).transpose(1, 2, 0)
    ).reshape(H_PER_CORE * D, LQ)
    return {"qT": qT, "kT": kT, "vT": vT}


def _run(q, k, v, k_cache, v_cache, block_tables, allow_mask,
         trace=False, tmpdir=None):
    from concourse.bass_utils import run_bass_kernel_spmd

    q = np.asarray(q, dtype=np.float32)
    k = np.asarray(k, dtype=np.float32)
    v = np.asarray(v, dtype=np.float32)
    k_cache = np.asarray(k_cache, dtype=np.float32)
    v_cache = np.asarray(v_cache, dtype=np.float32)
    block_tables = np.asarray(block_tables)

    qstarts = _derive_qstarts(allow_mask)
    nc = _get_nc(qstarts)
    in_maps = [_core_inputs(c, q, k, v, k_cache, v_cache, block_tables)
               for c in range(N_CORES)]
    res = run_bass_kernel_spmd(nc, in_maps, core_ids=list(range(N_CORES)),
                               trace=trace, tmpdir=tmpdir)

    out = np.empty((B, LQ, HQ, D), dtype=np.float32)
    for c in range(N_CORES):
        b, half = divmod(c, 2)
        oT = np.asarray(res.results[c]["outT"]).reshape(H_PER_CORE, D, LQ)
        dn = np.asarray(res.results[c]["den"])          # [H_PER_CORE, LQ]
        o = oT / dn[:, None, :]
        out[b, :, half * H_PER_CORE:(half + 1) * H_PER_CORE, :] = \
            o.transpose(2, 0, 1)
    return out, res


def kernel(q, k, v, k_cache, v_cache, block_tables, allow_mask):
    out, _ = _run(q, k, v, k_cache, v_cache, block_tables, allow_mask)
    return out
